# revision 58
# baseline (speedup 1.0000x reference)
"""Trainium2 Bass kernel for nn_CausalityMapBlock (raw bass, manual sync).

Math: with p = 1.0 and EPS = 1e-8 the whole block collapses to rank-1.
For xf = x/(max+EPS), S1 = sum(xf), S2 = sum(xf^2) per channel:
  lehmer_num[m,n] ~= S2[m]S2[n]/(S1[m]S1[n]),  lehmer_den[n] ~= S2[n]/S1[n]
  out[m,n] = lehmer_num/lehmer_den ~= S2[m]/S1[m]   (constant along n)
All EPS correction terms are O(1e-7) relative (verified 1.3e-6 max rel
err vs the reference).  With raw sums t = sum(x), q = sum(x^2):
  out[m,n] = q[m] / (t[m] * gmax)

Layout strategy: all per-channel math stays in [128,1] columns (128
lanes parallel). The partition axis is crossed exactly twice: one PE
transpose of the per-channel max column (for the global max) and one
K=1 N=1 bf16 matmul that broadcasts gmax back down the partitions
into PSUM (bf16 avoids the 4x fp32 PE penalty; 2e-3 rel). 1/gmax is
then a full-lane column reciprocal, and the final [128,128] result is
a single DVE tensor_scalar whose in0 is the u column read through a
stride-0 [128,128] AP with 1/gmax as the per-partition scalar. Row
ops on [1,p] tiles are single-lane (~7ns/elem) and are avoided
entirely except one reduce_max into a bf16 gmax.

Raw bass (no Tile framework): manual semaphores avoid Tile's startup
barrier and teardown sem-reset storm. Each instruction carries at most
one embedded wait (walrus limit); extra deps use standalone waits.

Sharding: data-parallel over batch B=2; cores 0-3 compute batch 0,
cores 4-7 batch 1 (redundantly within a group; wall-clock identical).
"""

import sys

import numpy as np

for _p in ("/opt/trn_rl_repo",):
    if _p not in sys.path:
        sys.path.insert(0, _p)

EPS = 1e-8
B, C, H, W = 2, 128, 7, 7
F = H * W  # 49
N_CORES = 8

_CACHE = {}


def _build_nc():
    import concourse.bacc as bacc
    import concourse.mybir as mybir

    fp32 = mybir.dt.float32
    MUL = mybir.AluOpType.mult
    AX = mybir.AxisListType.X
    COPY = mybir.ActivationFunctionType.Copy

    nc = bacc.Bacc("TRN2", target_bir_lowering=False, debug=False)
    xb = nc.dram_tensor("xb", [C, F], fp32, kind="ExternalInput")
    out = nc.dram_tensor("out", [C, C], fp32, kind="ExternalOutput")

    from contextlib import ExitStack

    with ExitStack() as ctx:
        sb = lambda name, shape, dt=fp32: ctx.enter_context(
            nc.sbuf_tensor(name, shape, dt)
        )
        ps = lambda name, shape: ctx.enter_context(
            nc.psum_tensor(name, shape, fp32)
        )
        ident = sb("ident", [128, 128])
        X = sb("X", [C, F])
        X2 = sb("X2", [C, F])      # DVE stt main output (q accum side)
        XJ = sb("XJ", [C, F])      # ACT copy main output (t accum side)
        mtc = sb("mtc", [C, 1])    # per-channel max column
        qcol = sb("qcol", [C, 1])  # q = sum(x^2) per channel
        tcol = sb("tcol", [C, 1])  # t = sum(x) per channel (from ACT)
        rtc = sb("rtc", [C, 1])    # 1/t column
        u0 = sb("u0", [C, 1])      # q/t column
        rgc = sb("rgc", [C, 1])    # 1/gmax column
        bf16 = mybir.dt.bfloat16
        onesr = sb("onesr", [1, 128], bf16)  # lhsT of the gmax broadcast
        gmax = sb("gmax", [1, 1], bf16)
        osb = sb("osb", [128, 128])
        jnk = sb("jnk", [1, 1])
        TPm = ps("TPm", [1, 128])   # transposed max column
        SVC = ps("SVC", [128, 1])   # gmax broadcast down partitions
        dma_sem = ctx.enter_context(nc.semaphore("dma_sem"))
        dve_sem = ctx.enter_context(nc.semaphore("dve_sem"))
        act_sem = ctx.enter_context(nc.semaphore("act_sem"))
        pe_sem = ctx.enter_context(nc.semaphore("pe_sem"))
        # input DMAs issue from the ENTRY basic block — right after the
        # framework preamble + pseudo-barrier, ~240ns before the block
        # bodies dispatch. Split across the two HWDGE queues (SP + ACT);
        # per-partition packet overhead dominates.
        # 96/32 row split: the SP queue's transfers consistently start
        # ~200ns before the ACT queue's, so SP takes the larger share
        # to equalize completion (d2d cost is partition-count-free)
        nc.sync.dma_start(X[0:96, :], xb.ap()[0:96, :]).then_inc(
            dma_sem, 16
        )
        nc.scalar.dma_start(X[96:128, :], xb.ap()[96:128, :]).then_inc(
            dma_sem, 16
        )
        block = ctx.enter_context(nc.Block(no_gpsimd_drain=True))

        @block.sync
        def _(sync):
            sync.wait_ge(dve_sem, 4)
            # no completion wait on the output DMAs: NRT drains the HWDGE
            # rings before signaling NEFF completion (incs required by
            # codegen; next run's preamble clears them)
            sync.dma_start(out.ap()[0:96, :], osb[0:96, :]).then_inc(
                dma_sem, 16
            )

        @block.scalar
        def _(scalar):
            # dummy activation: absorbs the one-time ACT table load while
            # the kernel is still waiting on the input DMA
            nc.scalar.copy(jnk[:], onesr[0:1, 0:1])._wait_ge(dve_sem, 1)
            # t = sum(x) per channel via Copy-with-accum, in parallel
            # with DVE's max/sumsq reduces
            nc.scalar.activation(
                XJ[:], X[:], COPY, accum_out=tcol[:]
            )._wait_ge(dma_sem, 32).then_inc(act_sem, 1)
            scalar.wait_ge(dve_sem, 4)
            scalar.dma_start(out.ap()[96:128, :], osb[96:128, :]).then_inc(
                dma_sem, 16
            )

        @block.gpsimd
        def _(gpsimd):
            # identity for the PE transpose; entirely off the critical
            # path (runs during the input-DMA wait)
            nc.gpsimd.memset(ident[:], 0.0)
            nc.gpsimd.drain()
            nc.gpsimd.affine_select(
                out=ident[:], in_=ident[:],
                compare_op=mybir.AluOpType.not_equal,
                fill=1.0, base=0,
                pattern=[[-1, 128]], channel_multiplier=1,
            ).then_inc(pe_sem, 1)

        @block.vector
        def _(vector):
            nc.vector.memset(onesr[:], 1.0).then_inc(dve_sem, 1)
            # per-channel stats (column layout, 128-lane parallel); the
            # transpose only needs mt, so it launches off this inc
            nc.vector.reduce_max(mtc[:], X[:], axis=AX)._wait_ge(
                dma_sem, 32
            ).then_inc(dve_sem, 1)  # dve=2 -> PE transpose go
            nc.vector.scalar_tensor_tensor(
                X2[:], X[:], 1.0, X[:], op0=MUL, op1=MUL,
                accum_out=qcol[:],
            )
            nc.vector.reciprocal(rtc[:], tcol[:])._wait_ge(act_sem, 1)
            # bf16 gmax (2e-3 rel) lets the broadcast matmul run at
            # bf16 speed; also the rtc->u0 RAW spacer
            nc.vector.reduce_max(gmax[:], TPm[:], axis=AX)._wait_ge(
                pe_sem, 2
            ).then_inc(dve_sem, 1)  # dve=3 -> PE gmax-broadcast go
            nc.vector.tensor_scalar_mul(u0[:], qcol[:], rtc[:])
            # 1/gmax on the broadcast column: full-lane reciprocal.
            # The out-DMA descriptor generation (~570ns, content-free)
            # launches off this inc and overlaps the drain + final copy
            # (~390ns): the doorbell still lands after osb completes,
            # before the ~700ns ring fetch even starts. (Gating one
            # producer earlier, on the broadcast matmul, was measured
            # WRONG — rel err 37 — do not push this further.)
            nc.vector.reciprocal(rgc[:], SVC[:])._wait_ge(
                pe_sem, 3
            ).then_inc(dve_sem, 1)  # dve=4 -> out-DMA descriptor gen go
            nc.vector.drain()
            # osb[m,n] = u0[m] * (1/gmax): u0 read as a stride-0
            # [128,128] AP with rgc as the per-partition scalar
            nc.vector.tensor_scalar_mul(
                osb[:], u0[:].broadcast_to([128, 128]), rgc[:]
            ).then_inc(dve_sem, 1)  # dve=5

        @block.tensor
        def _(tensor):
            tensor.wait_ge(pe_sem, 1)
            nc.tensor.transpose(TPm[:], mtc[:], ident[:])._wait_ge(
                dve_sem, 2
            ).then_inc(pe_sem, 1)
            # K=1 N=1 bf16 matmul broadcasts gmax down the partitions:
            # SVC[m,0] = onesr[0,m] * gmax
            nc.tensor.matmul(
                SVC[:], onesr[:], gmax[:], start=True, stop=True,
            )._wait_ge(dve_sem, 3).then_inc(pe_sem, 1)

    nc.compile()
    return nc


def _get_nc():
    if "nc" not in _CACHE:
        _CACHE["nc"] = _build_nc()
    return _CACHE["nc"]


def kernel(x) -> np.ndarray:
    from concourse.bass_utils import run_bass_kernel_spmd

    x = np.ascontiguousarray(np.asarray(x), dtype=np.float32)
    assert x.shape == (B, C, H, W)
    xf = x.reshape(B, C, F)

    nc = _get_nc()
    in_maps = [{"xb": np.ascontiguousarray(xf[i // 4])} for i in range(N_CORES)]
    try:
        res = run_bass_kernel_spmd(nc, in_maps, list(range(N_CORES))).results
    except Exception:
        # transient NRT/device hiccups recover on a clean retry
        res = run_bass_kernel_spmd(nc, in_maps, list(range(N_CORES))).results
    return np.stack([res[0]["out"], res[4]["out"]]).astype(np.float32)
